# revision 2
# baseline (speedup 1.0000x reference)
"""NodeSinkhornPooling kernel for 8 TRN2 NeuronCores.

Mathematical note (why this kernel is tiny):

The reference runs batched log-domain Sinkhorn and returns the *column
marginals* of the transport plan, normalized.  The iteration order in the
reference is `f = update(g); g = update(f)` — i.e. the **g-update (over
samples s) is applied last**.  By construction, after the g-update the
column marginals of P = exp((f+g-C)/eps + log_a + log_b) are *exactly*
the uniform target weights b_k = 1/K:

    sum_s P[s,k] = exp(g_k/eps + log_b) * sum_s exp((f_s - C_sk)/eps + log_a)
                 = exp(g_k/eps + log_b) * exp(-g_k/eps)  =  1/K ,

for every node, regardless of convergence.  The subsequent normalization
divides by sum_k 1/K = 1 (a no-op).  Hence the exact output of the
reference module is the constant 1/K everywhere.  Verified numerically:
float64 reference deviates from 1/K by ~3e-13 relative; the float32
reference deviates by ~1.5e-4 relative — pure f32 rounding noise (the
same order as *any* independent re-implementation, since the noise comes
from (f+g-C)/eps cancellations of O(100s)/0.09 magnitudes).

So the optimal kernel writes 1/K into the output.  We still run a real
SPMD Bass kernel across the 8 cores (sharded over the node dimension N,
matching the data-parallel hint): each core memsets its [N/8, K] shard
on-chip and DMAs it to its output buffer.
"""

import numpy as np

import concourse.bass as bass
import concourse.mybir as mybir
from concourse.bass_utils import run_bass_kernel_spmd

# Problem constants (hardcoded per contract; must match the grader's shapes).
N, S, D = 2048, 128, 256
K = 256
N_CORES = 8
NL = N // N_CORES  # 256 nodes per core

# Stashed result of the last device run (test.py reads exec_time_ns etc.).
LAST_RESULTS = None


def _build_nc() -> bass.Bass:
    nc = bass.Bass()
    out = nc.dram_tensor("hist", [NL, K], mybir.dt.float32, kind="ExternalOutput")

    with (
        nc.sbuf_tensor("t", [128, K], mybir.dt.float32) as t,
        nc.semaphore("val_sem") as val_sem,
        nc.semaphore("dma_sem") as dma_sem,
        nc.Block() as block,
    ):
        # Two HWDGE queues (sync + scalar) write the two 128-row halves in
        # parallel from the same memset tile.

        @block.vector
        def _(vector):
            vector.memset(t[:, :], 1.0 / K).then_inc(val_sem, 1)

        @block.scalar
        def _(scalar):
            scalar.wait_ge(val_sem, 1)
            scalar.dma_start(out=out[128:256, :], in_=t[:, :]).then_inc(dma_sem, 16)

        @block.sync
        def _(sync):
            sync.wait_ge(val_sem, 1)
            sync.dma_start(out=out[0:128, :], in_=t[:, :]).then_inc(dma_sem, 16)
            sync.wait_ge(dma_sem, 32)

    return nc


def kernel(samples: np.ndarray, codebook: np.ndarray) -> np.ndarray:
    global LAST_RESULTS
    assert samples.shape == (N, S, D), samples.shape
    assert codebook.shape == (K, D), codebook.shape

    nc = _build_nc()
    # Pure data-parallel over N; the output is input-independent, so the
    # shards carry no per-core input tensors.
    in_maps = [{} for _ in range(N_CORES)]
    res = run_bass_kernel_spmd(nc, in_maps, list(range(N_CORES)))
    LAST_RESULTS = res

    shards = [res.results[i]["hist"] for i in range(N_CORES)]
    return np.ascontiguousarray(np.concatenate(shards, axis=0), dtype=np.float32)
